# revision 1
# baseline (speedup 1.0000x reference)
"""Trainium2 Bass kernel for nn_CBAM_86947317940497 (CBAM-style gnn message passing).

Computation (N=100000 points, K=16 knn, C=64 ch, HID=16, 27-nbr sparse conv):
  g = x_F[idx]; gate = sigmoid(mlp(mean_k g) + mlp(max_k g)); outse = x_F*gate
  z = [mean_{k*c} outse[idx], max_{k*c} outse[idx]]
  convf = einsum(z[conv_idx]*mask, conv_w); out = outse * sigmoid(convf)

Distribution: points sharded 8 ways (12500/core, padded to 12544 = 98 tiles
of 128).  x_F replicated.  Three SPMD launches with host concat of the tiny
(N,2) stat tensors between launches:
  L1: knn gather (indirect DMA, 16 calls/tile) -> pool -> MLP gate -> outse,
      per-row mean/max stats sm.
  L2: gather sm pairs at idx -> z per point.
  L3: build zw[i*27+l] = z[i]*conv_w[l] table on device, gather pre-weighted
      pairs for up to 9 valid conv neighbors/point, reduce, sigmoid, multiply.
"""

from contextlib import ExitStack

import numpy as np

import concourse.bass as bass
import concourse.bacc as bacc
import concourse.mybir as mybir
from concourse.tile import TileContext
from concourse.bass_utils import run_bass_kernel_spmd
from concourse.masks import make_identity

N, K, C, HID = 100000, 16, 64, 16
NCORES = 8
SH = N // NCORES            # 12500 rows per core
P = 128
NT = (SH + P - 1) // P      # 98 tiles
SHP = NT * P                # 12544 padded rows
SMF_ROWS = NCORES * SHP     # 100352
ZF_ROWS = SMF_ROWS               # 100352 = 128*784
EMAX = 9                    # max valid conv neighbors per point (data has 9)

F32 = mybir.dt.float32
I32 = mybir.dt.int32


def _nc():
    return bacc.Bacc("TRN2", target_bir_lowering=False, debug=False,
                     num_devices=NCORES)


def build_l1(nt=NT, repeat=1):
    nc = _nc()
    xf = nc.dram_tensor("xf", [N, C], F32, kind="ExternalInput")
    xo = nc.dram_tensor("xo", [SHP, C], F32, kind="ExternalInput")
    ji = nc.dram_tensor("ji", [SHP, K], I32, kind="ExternalInput")
    w1 = nc.dram_tensor("w1", [C, HID], F32, kind="ExternalInput")
    b1 = nc.dram_tensor("b1", [HID, 1], F32, kind="ExternalInput")
    w2 = nc.dram_tensor("w2", [HID, C], F32, kind="ExternalInput")
    b2x2 = nc.dram_tensor("b2x2", [C, 1], F32, kind="ExternalInput")
    outse = nc.dram_tensor("outse", [SHP, C], F32, kind="ExternalOutput")
    sm = nc.dram_tensor("sm", [SHP, 2], F32, kind="ExternalOutput")

    with TileContext(nc) as tc:
        with tc.tile_pool(name="const", bufs=1) as cpool, \
             tc.tile_pool(name="sbuf", bufs=5) as pool, \
             tc.tile_pool(name="ipool", bufs=6) as ipool, \
             tc.tile_pool(name="psum", bufs=1, space="PSUM") as ppool:
            idt = cpool.tile([P, P], F32)
            make_identity(nc, idt[:])
            w1s = cpool.tile([C, HID], F32)
            nc.sync.dma_start(out=w1s[:], in_=w1[:])
            b1s = cpool.tile([HID, 1], F32)
            nc.sync.dma_start(out=b1s[:], in_=b1[:])
            w2s = cpool.tile([HID, C], F32)
            nc.sync.dma_start(out=w2s[:], in_=w2[:])
            b2s = cpool.tile([C, 1], F32)
            nc.sync.dma_start(out=b2s[:], in_=b2x2[:])

            rep_ctx = ExitStack()
            if repeat > 1:
                rep_ctx.enter_context(tc.For_i(0, repeat, 1))
            with rep_ctx, tc.For_i(0, nt * P, P) as r0:
                it = ipool.tile([P, K], I32)
                nc.sync.dma_start(out=it[:], in_=ji[bass.ds(r0, P), :])
                gt = pool.tile([P, K * C], F32, tag="g")
                for j in range(K):
                    nc.gpsimd.indirect_dma_start(
                        out=gt[:, j * C:(j + 1) * C], out_offset=None, in_=xf[:],
                        in_offset=bass.IndirectOffsetOnAxis(
                            ap=it[:, j:j + 1], axis=0),
                    )
                gv = gt[:].rearrange("p (j c) -> p c j", j=K)
                pm = pool.tile([P, C], F32, tag="pm")
                nc.vector.tensor_reduce(out=pm[:], in_=gv,
                                        axis=mybir.AxisListType.X,
                                        op=mybir.AluOpType.add)
                px = pool.tile([P, C], F32, tag="px")
                nc.vector.tensor_reduce(out=px[:], in_=gv,
                                        axis=mybir.AxisListType.X,
                                        op=mybir.AluOpType.max)
                # transpose both pools to [C, P]
                ps_m = ppool.tile([C, P], F32, tag="tp1")
                nc.tensor.transpose(out=ps_m[:], in_=pm[:], identity=idt[:])
                ps_x = ppool.tile([C, P], F32, tag="tp2")
                nc.tensor.transpose(out=ps_x[:], in_=px[:], identity=idt[:])
                poolT = pool.tile([C, 2 * P], F32, tag="poolT")
                # mean = sum/16 folded into the copy
                nc.scalar.activation(out=poolT[:, 0:P], in_=ps_m[:],
                                     func=mybir.ActivationFunctionType.Copy,
                                     scale=1.0 / K)
                nc.vector.tensor_copy(out=poolT[:, P:2 * P], in_=ps_x[:])
                ps1 = ppool.tile([HID, 2 * P], F32, tag="mm1")
                nc.tensor.matmul(out=ps1[:], lhsT=w1s[:], rhs=poolT[:],
                                 start=True, stop=True)
                h = pool.tile([HID, 2 * P], F32, tag="h")
                nc.scalar.activation(out=h[:], in_=ps1[:],
                                     func=mybir.ActivationFunctionType.Relu,
                                     bias=b1s[:])
                ps2 = ppool.tile([C, 2 * P], F32, tag="mm2")
                nc.tensor.matmul(out=ps2[:], lhsT=w2s[:], rhs=h[:],
                                 start=True, stop=True)
                g2 = pool.tile([C, 2 * P], F32, tag="g2")
                nc.vector.tensor_copy(out=g2[:], in_=ps2[:])
                pre = pool.tile([C, P], F32, tag="pre")
                nc.vector.tensor_add(out=pre[:], in0=g2[:, 0:P],
                                     in1=g2[:, P:2 * P])
                gT = pool.tile([C, P], F32, tag="gT")
                nc.scalar.activation(out=gT[:], in_=pre[:],
                                     func=mybir.ActivationFunctionType.Sigmoid,
                                     bias=b2s[:])
                psg = ppool.tile([P, C], F32, tag="tpg")
                nc.tensor.transpose(out=psg[:], in_=gT[:],
                                    identity=idt[0:C, 0:C])
                gate = pool.tile([P, C], F32, tag="gate")
                nc.vector.tensor_copy(out=gate[:], in_=psg[:])
                xt = pool.tile([P, C], F32, tag="xt")
                nc.sync.dma_start(out=xt[:], in_=xo[bass.ds(r0, P), :])
                ot = pool.tile([P, C], F32, tag="ot")
                nc.vector.tensor_mul(out=ot[:], in0=xt[:], in1=gate[:])
                nc.sync.dma_start(out=outse[bass.ds(r0, P), :], in_=ot[:])
                smt = pool.tile([P, 2], F32, tag="smt")
                s0 = pool.tile([P, 1], F32, tag="s0")
                nc.vector.tensor_reduce(out=s0[:], in_=ot[:],
                                        axis=mybir.AxisListType.X,
                                        op=mybir.AluOpType.add)
                nc.scalar.activation(out=smt[:, 0:1], in_=s0[:],
                                     func=mybir.ActivationFunctionType.Copy,
                                     scale=1.0 / C)
                nc.vector.tensor_reduce(out=smt[:, 1:2], in_=ot[:],
                                        axis=mybir.AxisListType.X,
                                        op=mybir.AluOpType.max)
                nc.sync.dma_start(out=sm[bass.ds(r0, P), :], in_=smt[:])
    nc.compile()
    return nc


def build_l2(repeat=1):
    nc = _nc()
    smf = nc.dram_tensor("smf", [SMF_ROWS, 2], F32, kind="ExternalInput")
    ji2 = nc.dram_tensor("ji2", [SHP, K], I32, kind="ExternalInput")
    z = nc.dram_tensor("z", [SHP, 2], F32, kind="ExternalOutput")
    with TileContext(nc) as tc:
        with tc.tile_pool(name="sbuf", bufs=6) as pool, \
             tc.tile_pool(name="ipool", bufs=6) as ipool:
            rep_ctx = ExitStack()
            if repeat > 1:
                rep_ctx.enter_context(tc.For_i(0, repeat, 1))
            with rep_ctx, tc.For_i(0, NT * P, P) as r0:
                it = ipool.tile([P, K], I32)
                nc.sync.dma_start(out=it[:], in_=ji2[bass.ds(r0, P), :])
                sg = pool.tile([P, K * 2], F32, tag="sg")
                for j in range(K):
                    nc.gpsimd.indirect_dma_start(
                        out=sg[:, j * 2:(j + 1) * 2], out_offset=None, in_=smf[:],
                        in_offset=bass.IndirectOffsetOnAxis(
                            ap=it[:, j:j + 1], axis=0),
                    )
                sv = sg[:].rearrange("p (j c) -> p c j", j=K)
                rs = pool.tile([P, 2], F32, tag="rs")
                nc.vector.tensor_reduce(out=rs[:], in_=sv,
                                        axis=mybir.AxisListType.X,
                                        op=mybir.AluOpType.add)
                rm = pool.tile([P, 2], F32, tag="rm")
                nc.vector.tensor_reduce(out=rm[:], in_=sv,
                                        axis=mybir.AxisListType.X,
                                        op=mybir.AluOpType.max)
                zt = pool.tile([P, 2], F32, tag="zt")
                nc.scalar.activation(out=zt[:, 0:1], in_=rs[:, 0:1],
                                     func=mybir.ActivationFunctionType.Copy,
                                     scale=1.0 / K)
                nc.vector.tensor_copy(out=zt[:, 1:2], in_=rm[:, 1:2])
                nc.sync.dma_start(out=z[bass.ds(r0, P), :], in_=zt[:])
    nc.compile()
    return nc


def build_l3(repeat=1, emax=EMAX):
    nc = _nc()
    zf = nc.dram_tensor("zf", [ZF_ROWS, 2], F32, kind="ExternalInput")
    cwt = nc.dram_tensor("cwt", [32, 2], F32, kind="ExternalInput")
    ei2 = nc.dram_tensor("ei2", [SHP, 2 * emax], I32, kind="ExternalInput")
    oi = nc.dram_tensor("oi", [SHP, C], F32, kind="ExternalInput")
    out = nc.dram_tensor("out", [SHP, C], F32, kind="ExternalOutput")

    with TileContext(nc) as tc:
        with tc.tile_pool(name="sbuf", bufs=6) as pool, \
             tc.tile_pool(name="ipool", bufs=6) as ipool:
            rep_ctx = ExitStack()
            if repeat > 1:
                rep_ctx.enter_context(tc.For_i(0, repeat, 1))
            with rep_ctx, tc.For_i(0, NT * P, P) as r0:
                et = ipool.tile([P, 2 * emax], I32)
                nc.sync.dma_start(out=et[:], in_=ei2[bass.ds(r0, P), :])
                zn = pool.tile([P, emax * 2], F32, tag="zn")
                wn = pool.tile([P, emax * 2], F32, tag="wn")
                for e in range(emax):
                    nc.gpsimd.indirect_dma_start(
                        out=zn[:, e * 2:(e + 1) * 2], out_offset=None, in_=zf[:],
                        in_offset=bass.IndirectOffsetOnAxis(
                            ap=et[:, e:e + 1], axis=0),
                    )
                    nc.gpsimd.indirect_dma_start(
                        out=wn[:, e * 2:(e + 1) * 2], out_offset=None, in_=cwt[:],
                        in_offset=bass.IndirectOffsetOnAxis(
                            ap=et[:, emax + e:emax + e + 1], axis=0),
                    )
                pr = pool.tile([P, emax * 2], F32, tag="pr")
                nc.vector.tensor_mul(out=pr[:], in0=zn[:], in1=wn[:])
                cf = pool.tile([P, 1], F32, tag="cf")
                nc.vector.tensor_reduce(
                    out=cf[:], in_=pr[:],
                    axis=mybir.AxisListType.X, op=mybir.AluOpType.add)
                sg = pool.tile([P, 1], F32, tag="sig")
                nc.scalar.activation(out=sg[:], in_=cf[:],
                                     func=mybir.ActivationFunctionType.Sigmoid)
                ot = pool.tile([P, C], F32, tag="ot")
                nc.sync.dma_start(out=ot[:], in_=oi[bass.ds(r0, P), :])
                ft = pool.tile([P, C], F32, tag="ft")
                nc.vector.tensor_mul(out=ft[:], in0=ot[:],
                                     in1=sg[:].to_broadcast([P, C]))
                nc.sync.dma_start(out=out[bass.ds(r0, P), :], in_=ft[:])
    nc.compile()
    return nc


def _pad_rows(a, rows):
    out = np.zeros((rows,) + a.shape[1:], a.dtype)
    out[:a.shape[0]] = a
    return out


def kernel(x_F, W1, b1, W2, b2, conv_w, idx, conv_idx):
    x_F = np.ascontiguousarray(np.asarray(x_F, dtype=np.float32))
    W1 = np.asarray(W1, dtype=np.float32)
    b1 = np.asarray(b1, dtype=np.float32)
    W2 = np.asarray(W2, dtype=np.float32)
    b2 = np.asarray(b2, dtype=np.float32)
    conv_w = np.asarray(conv_w, dtype=np.float32)
    idx = np.asarray(idx).astype(np.int32)
    conv_idx = np.asarray(conv_idx).astype(np.int32)

    cores = list(range(NCORES))
    # host-side index prep (pure index transforms)
    qmap = lambda n: (n // SH) * SHP + (n % SH)          # noqa: E731
    idx_q = qmap(idx.astype(np.int64)).astype(np.int32)  # shard-concat ids
    valid = conv_idx >= 0
    ci_q = np.where(valid, qmap(np.clip(conv_idx, 0, None).astype(np.int64)),
                    0).astype(np.int32)
    lidx = np.broadcast_to(np.arange(27, dtype=np.int32), conv_idx.shape)
    lrow = np.where(valid, lidx, 27).astype(np.int32)    # 27 -> zero weight row
    # compact valid entries to the first emax slots per row
    cnt = valid.sum(1)
    emax = max(int(cnt.max()), 1)
    order = np.argsort(~valid, axis=1, kind="stable")    # valid first
    eiz = np.take_along_axis(ci_q, order, axis=1)[:, :emax]
    eil = np.take_along_axis(lrow, order, axis=1)[:, :emax]
    ei2 = np.ascontiguousarray(np.concatenate([eiz, eil], axis=1))

    # ---- L1
    nc1 = build_l1()
    in1 = []
    for c in cores:
        sl = slice(c * SH, (c + 1) * SH)
        in1.append({
            "xf": x_F,
            "xo": _pad_rows(x_F[sl], SHP),
            "ji": _pad_rows(idx[sl], SHP),
            "w1": W1,
            "b1": b1.reshape(HID, 1),
            "w2": W2,
            "b2x2": (2.0 * b2).reshape(C, 1),
        })
    r1 = run_bass_kernel_spmd(nc1, in1, core_ids=cores)
    outse = np.concatenate([r1.results[c]["outse"] for c in cores], 0)
    smf = np.concatenate([r1.results[c]["sm"] for c in cores], 0)

    # ---- L2
    nc2 = build_l2()
    in2 = []
    for c in cores:
        sl = slice(c * SH, (c + 1) * SH)
        in2.append({"smf": smf, "ji2": _pad_rows(idx_q[sl], SHP)})
    r2 = run_bass_kernel_spmd(nc2, in2, core_ids=cores)
    zf = np.zeros((ZF_ROWS, 2), np.float32)
    for c in cores:
        zc = r2.results[c]["z"]
        zf[c * SHP: c * SHP + SH] = zc[:SH]   # zero the pad rows
    # ---- L3
    nc3 = build_l3(emax=emax)
    cwt = np.zeros((32, 2), np.float32)
    cwt[:27] = conv_w.reshape(27, 2)
    in3 = []
    for c in cores:
        sl = slice(c * SH, (c + 1) * SH)
        in3.append({
            "zf": zf,
            "cwt": cwt,
            "ei2": _pad_rows(ei2[sl], SHP),
            "oi": outse[c * SHP:(c + 1) * SHP],
        })
    r3 = run_bass_kernel_spmd(nc3, in3, core_ids=cores)
    out = np.concatenate([r3.results[c]["out"][:SH] for c in cores], 0)
    return out



# revision 8
# speedup vs baseline: 1.0161x; 1.0161x over previous
"""Trainium2 Bass kernel for nn_CBAM_86947317940497 (CBAM-style gnn message passing).

Computation (N=100000 points, K=16 knn, C=64 ch, HID=16, 27-nbr sparse conv):
  g = x_F[idx]; gate = sigmoid(mlp(mean_k g) + mlp(max_k g)); outse = x_F*gate
  z = [mean_{k*c} outse[idx], max_{k*c} outse[idx]]
  convf = einsum(z[conv_idx]*mask, conv_w); out = outse * sigmoid(convf)

Distribution: points sharded 8 ways (12500/core, padded to 12544 = 98 tiles
of 128).  x_F replicated.  Three SPMD launches with host index prep between
them.  All loops are fully unrolled python loops (no per-iteration all-engine
barriers), so the SWDGE indirect-gather calls stream back-to-back on the Pool
engine and all other engine work pipelines underneath them.
"""

from contextlib import ExitStack

import numpy as np

import concourse.bass as bass
import concourse.bacc as bacc
import concourse.mybir as mybir
from concourse.tile import TileContext
from concourse.bass_utils import run_bass_kernel_spmd
from concourse.masks import make_identity

N, K, C, HID = 100000, 16, 64, 16
NCORES = 8
SH = N // NCORES            # 12500 rows per core
P = 128
NT = (SH + P - 1) // P      # 98 tiles
SHP = NT * P                # 12544 padded rows
SMF_ROWS = NCORES * SHP     # 100352
ZF_ROWS = SMF_ROWS
EMAX = 9                    # max valid conv neighbors per point (data has 9)

F32 = mybir.dt.float32
I32 = mybir.dt.int32

LAST_TIMES = {}


def _nc():
    return bacc.Bacc("TRN2", target_bir_lowering=False, debug=False,
                     num_devices=NCORES)


def _run(nc, in_maps, label=None):
    return run_bass_kernel_spmd(nc, in_maps, core_ids=list(range(NCORES)))


def build_l1(repeat=1):
    nc = _nc()
    xf = nc.dram_tensor("xf", [N, C], F32, kind="ExternalInput")
    xo = nc.dram_tensor("xo", [SHP, C], F32, kind="ExternalInput")
    ji = nc.dram_tensor("ji", [SHP, K], I32, kind="ExternalInput")
    w1 = nc.dram_tensor("w1", [C, HID], F32, kind="ExternalInput")
    b1 = nc.dram_tensor("b1", [HID, 1], F32, kind="ExternalInput")
    w2 = nc.dram_tensor("w2", [HID, C], F32, kind="ExternalInput")
    b2x2 = nc.dram_tensor("b2x2", [C, 1], F32, kind="ExternalInput")
    outse = nc.dram_tensor("outse", [SHP, C], F32, kind="ExternalOutput")
    sm = nc.dram_tensor("sm", [SHP, 2], F32, kind="ExternalOutput")

    KC = K * C
    with TileContext(nc) as tc:
        with tc.tile_pool(name="const", bufs=1) as cpool, \
             tc.tile_pool(name="sbuf", bufs=3) as pool, \
             tc.tile_pool(name="ipool", bufs=3) as ipool, \
             tc.tile_pool(name="psum", bufs=1, space="PSUM") as ppool, \
             tc.tile_pool(name="psum2", bufs=2, space="PSUM") as ppool2:
            idt = cpool.tile([P, P], F32)
            make_identity(nc, idt[:])
            w1s = cpool.tile([C, HID], F32)
            nc.sync.dma_start(out=w1s[:], in_=w1[:])
            b1s = cpool.tile([HID, 1], F32)
            nc.sync.dma_start(out=b1s[:], in_=b1[:])
            w2s = cpool.tile([HID, C], F32)
            nc.sync.dma_start(out=w2s[:], in_=w2[:])
            b2s = cpool.tile([C, 1], F32)
            nc.sync.dma_start(out=b2s[:], in_=b2x2[:])

            rep_ctx = ExitStack()
            if repeat > 1:
                rep_ctx.enter_context(tc.For_i(0, repeat, 1))
            with rep_ctx:
                for t in range(NT):
                    r0 = t * P
                    it = ipool.tile([P, K], I32, tag="it")
                    nc.sync.dma_start(out=it[:], in_=ji[r0:r0 + P, :])
                    gt = pool.tile([P, KC], F32, tag="g")
                    for j in range(K):
                        nc.gpsimd.indirect_dma_start(
                            out=gt[:, j * C:(j + 1) * C], out_offset=None,
                            in_=xf[:],
                            in_offset=bass.IndirectOffsetOnAxis(
                                ap=it[:, j:j + 1], axis=0),
                        )
                    gv = gt[:].rearrange("p (j c) -> p c j", j=K)
                    pm = pool.tile([P, C], F32, tag="pm")
                    nc.vector.tensor_reduce(out=pm[:], in_=gv,
                                            axis=mybir.AxisListType.X,
                                            op=mybir.AluOpType.add)
                    px = pool.tile([P, C], F32, tag="px")
                    nc.vector.tensor_reduce(out=px[:], in_=gv,
                                            axis=mybir.AxisListType.X,
                                            op=mybir.AluOpType.max)
                    # transpose both pools to [C, P]
                    ps_m = ppool.tile([C, P], F32, tag="tp1")
                    nc.tensor.transpose(out=ps_m[:], in_=pm[:], identity=idt[:])
                    ps_x = ppool.tile([C, P], F32, tag="tp2")
                    nc.tensor.transpose(out=ps_x[:], in_=px[:], identity=idt[:])
                    poolT = pool.tile([C, 2 * P], F32, tag="poolT")
                    # mean = sum/16 folded into the copy
                    nc.scalar.activation(out=poolT[:, 0:P], in_=ps_m[:],
                                         func=mybir.ActivationFunctionType.Copy,
                                         scale=1.0 / K)
                    nc.scalar.activation(out=poolT[:, P:2 * P], in_=ps_x[:],
                                         func=mybir.ActivationFunctionType.Copy)
                    ps1 = ppool2.tile([HID, 2 * P], F32, tag="mm1")
                    nc.tensor.matmul(out=ps1[:], lhsT=w1s[:], rhs=poolT[:],
                                     start=True, stop=True)
                    h = pool.tile([HID, 2 * P], F32, tag="h")
                    nc.scalar.activation(out=h[:], in_=ps1[:],
                                         func=mybir.ActivationFunctionType.Relu,
                                         bias=b1s[:])
                    # accumulate mean- and max-path MLP outputs directly in PSUM
                    ps2 = ppool2.tile([C, P], F32, tag="mm2")
                    nc.tensor.matmul(out=ps2[:], lhsT=w2s[:], rhs=h[:, 0:P],
                                     start=True, stop=False)
                    nc.tensor.matmul(out=ps2[:], lhsT=w2s[:], rhs=h[:, P:2 * P],
                                     start=False, stop=True)
                    gT = pool.tile([C, P], F32, tag="gT")
                    nc.scalar.activation(out=gT[:], in_=ps2[:],
                                         func=mybir.ActivationFunctionType.Sigmoid,
                                         bias=b2s[:])
                    psg = ppool.tile([P, C], F32, tag="tpg")
                    nc.tensor.transpose(out=psg[:], in_=gT[:],
                                        identity=idt[0:C, 0:C])
                    gate = pool.tile([P, C], F32, tag="gate")
                    nc.scalar.activation(out=gate[:], in_=psg[:],
                                         func=mybir.ActivationFunctionType.Copy)
                    xt = pool.tile([P, C], F32, tag="xt")
                    nc.sync.dma_start(out=xt[:], in_=xo[r0:r0 + P, :])
                    ot = pool.tile([P, C], F32, tag="ot")
                    s0 = pool.tile([P, 1], F32, tag="s0")
                    nc.vector.tensor_mul(out=ot[:], in0=xt[:], in1=gate[:])
                    nc.vector.tensor_reduce(out=s0[:], in_=ot[:],
                                            axis=mybir.AxisListType.X,
                                            op=mybir.AluOpType.add)
                    nc.sync.dma_start(out=outse[r0:r0 + P, :], in_=ot[:])
                    smt = pool.tile([P, 2], F32, tag="smt")
                    nc.scalar.activation(out=smt[:, 0:1], in_=s0[:],
                                         func=mybir.ActivationFunctionType.Copy,
                                         scale=1.0 / C)
                    nc.vector.tensor_reduce(out=smt[:, 1:2], in_=ot[:],
                                            axis=mybir.AxisListType.X,
                                            op=mybir.AluOpType.max)
                    nc.sync.dma_start(out=sm[r0:r0 + P, :], in_=smt[:])
    nc.compile()
    return nc


def build_l2(repeat=1):
    nc = _nc()
    smf = nc.dram_tensor("smf", [SMF_ROWS, 2], F32, kind="ExternalInput")
    ji2 = nc.dram_tensor("ji2", [SHP, K], I32, kind="ExternalInput")
    z = nc.dram_tensor("z", [SHP, 2], F32, kind="ExternalOutput")
    with TileContext(nc) as tc:
        with tc.tile_pool(name="sbuf", bufs=4) as pool, \
             tc.tile_pool(name="ipool", bufs=4) as ipool:
            rep_ctx = ExitStack()
            if repeat > 1:
                rep_ctx.enter_context(tc.For_i(0, repeat, 1))
            with rep_ctx:
                for t in range(NT):
                    r0 = t * P
                    it = ipool.tile([P, K], I32, tag="it")
                    nc.sync.dma_start(out=it[:], in_=ji2[r0:r0 + P, :])
                    sg = pool.tile([P, K * 2], F32, tag="sg")
                    for j in range(K):
                        nc.gpsimd.indirect_dma_start(
                            out=sg[:, j * 2:(j + 1) * 2], out_offset=None,
                            in_=smf[:],
                            in_offset=bass.IndirectOffsetOnAxis(
                                ap=it[:, j:j + 1], axis=0),
                        )
                    sv = sg[:].rearrange("p (j c) -> p c j", j=K)
                    rs = pool.tile([P, 2], F32, tag="rs")
                    nc.vector.tensor_reduce(out=rs[:], in_=sv,
                                            axis=mybir.AxisListType.X,
                                            op=mybir.AluOpType.add)
                    rm = pool.tile([P, 2], F32, tag="rm")
                    nc.vector.tensor_reduce(out=rm[:], in_=sv,
                                            axis=mybir.AxisListType.X,
                                            op=mybir.AluOpType.max)
                    zt = pool.tile([P, 2], F32, tag="zt")
                    nc.scalar.activation(out=zt[:, 0:1], in_=rs[:, 0:1],
                                         func=mybir.ActivationFunctionType.Copy,
                                         scale=1.0 / K)
                    nc.vector.tensor_copy(out=zt[:, 1:2], in_=rm[:, 1:2])
                    nc.sync.dma_start(out=z[r0:r0 + P, :], in_=zt[:])
    nc.compile()
    return nc


def build_l3(repeat=1, emax=EMAX):
    nc = _nc()
    zf = nc.dram_tensor("zf", [ZF_ROWS, 2], F32, kind="ExternalInput")
    cwt = nc.dram_tensor("cwt", [32, 2], F32, kind="ExternalInput")
    ei2 = nc.dram_tensor("ei2", [SHP, 2 * emax], I32, kind="ExternalInput")
    oi = nc.dram_tensor("oi", [SHP, C], F32, kind="ExternalInput")
    out = nc.dram_tensor("out", [SHP, C], F32, kind="ExternalOutput")

    with TileContext(nc) as tc:
        with tc.tile_pool(name="sbuf", bufs=4) as pool, \
             tc.tile_pool(name="ipool", bufs=4) as ipool:
            rep_ctx = ExitStack()
            if repeat > 1:
                rep_ctx.enter_context(tc.For_i(0, repeat, 1))
            with rep_ctx:
                for t in range(NT):
                    r0 = t * P
                    et = ipool.tile([P, 2 * emax], I32, tag="et")
                    nc.sync.dma_start(out=et[:], in_=ei2[r0:r0 + P, :])
                    zn = pool.tile([P, emax * 2], F32, tag="zn")
                    wn = pool.tile([P, emax * 2], F32, tag="wn")
                    for e in range(emax):
                        nc.gpsimd.indirect_dma_start(
                            out=zn[:, e * 2:(e + 1) * 2], out_offset=None,
                            in_=zf[:],
                            in_offset=bass.IndirectOffsetOnAxis(
                                ap=et[:, e:e + 1], axis=0),
                        )
                        nc.gpsimd.indirect_dma_start(
                            out=wn[:, e * 2:(e + 1) * 2], out_offset=None,
                            in_=cwt[:],
                            in_offset=bass.IndirectOffsetOnAxis(
                                ap=et[:, emax + e:emax + e + 1], axis=0),
                        )
                    pr = pool.tile([P, emax * 2], F32, tag="pr")
                    cf = pool.tile([P, 1], F32, tag="cf")
                    nc.vector.tensor_mul(out=pr[:], in0=zn[:], in1=wn[:])
                    nc.vector.tensor_reduce(out=cf[:], in_=pr[:],
                                            axis=mybir.AxisListType.X,
                                            op=mybir.AluOpType.add)
                    sg = pool.tile([P, 1], F32, tag="sig")
                    nc.scalar.activation(out=sg[:], in_=cf[:],
                                         func=mybir.ActivationFunctionType.Sigmoid)
                    ot = pool.tile([P, C], F32, tag="ot")
                    nc.sync.dma_start(out=ot[:], in_=oi[r0:r0 + P, :])
                    ft = pool.tile([P, C], F32, tag="ft")
                    nc.vector.tensor_mul(out=ft[:], in0=ot[:],
                                         in1=sg[:].to_broadcast([P, C]))
                    nc.sync.dma_start(out=out[r0:r0 + P, :], in_=ft[:])
    nc.compile()
    return nc


def _pad_rows(a, rows):
    out = np.zeros((rows,) + a.shape[1:], a.dtype)
    out[:a.shape[0]] = a
    return out


def kernel(x_F, W1, b1, W2, b2, conv_w, idx, conv_idx):
    x_F = np.ascontiguousarray(np.asarray(x_F, dtype=np.float32))
    W1 = np.asarray(W1, dtype=np.float32)
    b1 = np.asarray(b1, dtype=np.float32)
    W2 = np.asarray(W2, dtype=np.float32)
    b2 = np.asarray(b2, dtype=np.float32)
    conv_w = np.asarray(conv_w, dtype=np.float32)
    idx = np.asarray(idx).astype(np.int32)
    conv_idx = np.asarray(conv_idx).astype(np.int32)

    cores = list(range(NCORES))
    # host-side index prep (pure index transforms)
    qmap = lambda n: (n // SH) * SHP + (n % SH)          # noqa: E731
    idx_q = qmap(idx.astype(np.int64)).astype(np.int32)  # shard-concat ids
    valid = conv_idx >= 0
    ci_q = np.where(valid, qmap(np.clip(conv_idx, 0, None).astype(np.int64)),
                    0).astype(np.int32)
    lidx = np.broadcast_to(np.arange(27, dtype=np.int32), conv_idx.shape)
    lrow = np.where(valid, lidx, 27).astype(np.int32)    # 27 -> zero weight row
    # compact valid entries to the first emax slots per row
    cnt = valid.sum(1)
    emax = max(int(cnt.max()), 1)
    order = np.argsort(~valid, axis=1, kind="stable")    # valid first
    eiz = np.take_along_axis(ci_q, order, axis=1)[:, :emax]
    eil = np.take_along_axis(lrow, order, axis=1)[:, :emax]
    ei2 = np.ascontiguousarray(np.concatenate([eiz, eil], axis=1))

    # ---- L1
    nc1 = build_l1()
    in1 = []
    for c in cores:
        sl = slice(c * SH, (c + 1) * SH)
        in1.append({
            "xf": x_F,
            "xo": _pad_rows(x_F[sl], SHP),
            "ji": _pad_rows(idx[sl], SHP),
            "w1": W1,
            "b1": b1.reshape(HID, 1),
            "w2": W2,
            "b2x2": (2.0 * b2).reshape(C, 1),
        })
    r1 = _run(nc1, in1, "L1")
    outse = np.concatenate([r1.results[c]["outse"] for c in cores], 0)
    smf = np.concatenate([r1.results[c]["sm"] for c in cores], 0)

    # ---- L2
    nc2 = build_l2()
    in2 = []
    for c in cores:
        sl = slice(c * SH, (c + 1) * SH)
        in2.append({"smf": smf, "ji2": _pad_rows(idx_q[sl], SHP)})
    r2 = _run(nc2, in2, "L2")
    zf = np.zeros((ZF_ROWS, 2), np.float32)
    for c in cores:
        zc = r2.results[c]["z"]
        zf[c * SHP: c * SHP + SH] = zc[:SH]   # zero the pad rows

    # ---- L3
    nc3 = build_l3(emax=emax)
    cwt = np.zeros((32, 2), np.float32)
    cwt[:27] = conv_w.reshape(27, 2)
    in3 = []
    for c in cores:
        sl = slice(c * SH, (c + 1) * SH)
        in3.append({
            "zf": zf,
            "cwt": cwt,
            "ei2": _pad_rows(ei2[sl], SHP),
            "oi": outse[c * SHP:(c + 1) * SHP],
        })
    r3 = _run(nc3, in3, "L3")
    out = np.concatenate([r3.results[c]["out"][:SH] for c in cores], 0)
    return out


# revision 11
# speedup vs baseline: 2.2407x; 2.2053x over previous
"""Trainium2 Bass kernel for nn_CBAM_86947317940497 (CBAM-style gnn message passing).

Computation (N=100000 points, K=16 knn, C=64 ch, HID=16, 27-nbr sparse conv):
  g = x_F[idx]; gate = sigmoid(mlp(mean_k g) + mlp(max_k g)); outse = x_F*gate
  z = [mean_{k*c} outse[idx], max_{k*c} outse[idx]]
  convf = einsum(z[conv_idx]*mask, conv_w); out = outse * sigmoid(convf)

Distribution: points sharded 8 ways (12500/core, padded to 12544 = 98 tiles
of 128).  x_F replicated.  Three SPMD launches with host index prep between
them.  All loops are fully unrolled python loops (no per-iteration all-engine
barriers), so the SWDGE indirect-gather calls stream back-to-back on the Pool
engine and all other engine work pipelines underneath them.
"""

from contextlib import ExitStack

import numpy as np

import concourse.bass as bass
import concourse.bacc as bacc
import concourse.mybir as mybir
from concourse.tile import TileContext
from concourse.bass_utils import run_bass_kernel_spmd
from concourse.masks import make_identity

N, K, C, HID = 100000, 16, 64, 16
NCORES = 8
SH = N // NCORES            # 12500 rows per core
P = 128
NT = (SH + P - 1) // P      # 98 tiles
SHP = NT * P                # 12544 padded rows
SMF_ROWS = NCORES * SHP     # 100352
ZF_ROWS = SMF_ROWS
EMAX = 9                    # max valid conv neighbors per point (data has 9)

F32 = mybir.dt.float32
BF16 = mybir.dt.bfloat16
I32 = mybir.dt.int32

LAST_TIMES = {}


def _nc():
    return bacc.Bacc("TRN2", target_bir_lowering=False, debug=False,
                     num_devices=NCORES)


def _run(nc, in_maps, label=None):
    return run_bass_kernel_spmd(nc, in_maps, core_ids=list(range(NCORES)))


def build_l1(repeat=1):
    nc = _nc()
    xf = nc.dram_tensor("xf", [N, C], BF16, kind="ExternalInput")
    xo = nc.dram_tensor("xo", [SHP, C], F32, kind="ExternalInput")
    ji = nc.dram_tensor("ji", [SHP, K], I32, kind="ExternalInput")
    w1 = nc.dram_tensor("w1", [C, HID], F32, kind="ExternalInput")
    b1 = nc.dram_tensor("b1", [HID, 1], F32, kind="ExternalInput")
    w2 = nc.dram_tensor("w2", [HID, C], F32, kind="ExternalInput")
    b2x2 = nc.dram_tensor("b2x2", [C, 1], F32, kind="ExternalInput")
    outse = nc.dram_tensor("outse", [SHP, C], F32, kind="ExternalOutput")
    sm = nc.dram_tensor("sm", [SHP, 2], BF16, kind="ExternalOutput")

    KC = K * C
    with TileContext(nc) as tc:
        with tc.tile_pool(name="const", bufs=1) as cpool, \
             tc.tile_pool(name="sbuf", bufs=3) as pool, \
             tc.tile_pool(name="ipool", bufs=3) as ipool, \
             tc.tile_pool(name="psum", bufs=1, space="PSUM") as ppool, \
             tc.tile_pool(name="psum2", bufs=2, space="PSUM") as ppool2:
            idt = cpool.tile([P, P], F32)
            make_identity(nc, idt[:])
            w1s = cpool.tile([C, HID], F32)
            nc.sync.dma_start(out=w1s[:], in_=w1[:])
            b1s = cpool.tile([HID, 1], F32)
            nc.sync.dma_start(out=b1s[:], in_=b1[:])
            w2s = cpool.tile([HID, C], F32)
            nc.sync.dma_start(out=w2s[:], in_=w2[:])
            b2s = cpool.tile([C, 1], F32)
            nc.sync.dma_start(out=b2s[:], in_=b2x2[:])

            rep_ctx = ExitStack()
            if repeat > 1:
                rep_ctx.enter_context(tc.For_i(0, repeat, 1))
            with rep_ctx:
                for t in range(NT):
                    r0 = t * P
                    it = ipool.tile([P, K], I32, tag="it")
                    nc.sync.dma_start(out=it[:], in_=ji[r0:r0 + P, :])
                    gt = pool.tile([P, KC], BF16, tag="g")
                    for j in range(K):
                        nc.gpsimd.indirect_dma_start(
                            out=gt[:, j * C:(j + 1) * C], out_offset=None,
                            in_=xf[:],
                            in_offset=bass.IndirectOffsetOnAxis(
                                ap=it[:, j:j + 1], axis=0),
                        )
                    # contiguous pairwise trees (no strided SBUF reads,
                    # which contend with the concurrent gather writes)
                    with nc.allow_low_precision("bf16 knn pooling"):
                        s1 = pool.tile([P, KC // 2], BF16, tag="s1")
                        nc.vector.tensor_add(out=s1[:], in0=gt[:, 0:KC // 2],
                                             in1=gt[:, KC // 2:KC])
                        m1 = pool.tile([P, KC // 2], BF16, tag="m1")
                        nc.vector.tensor_tensor(out=m1[:], in0=gt[:, 0:KC // 2],
                                                in1=gt[:, KC // 2:KC],
                                                op=mybir.AluOpType.max)
                        s2 = pool.tile([P, KC // 4], BF16, tag="s2")
                        nc.vector.tensor_add(out=s2[:], in0=s1[:, 0:KC // 4],
                                             in1=s1[:, KC // 4:KC // 2])
                        m2 = pool.tile([P, KC // 4], BF16, tag="m2")
                        nc.vector.tensor_tensor(out=m2[:], in0=m1[:, 0:KC // 4],
                                                in1=m1[:, KC // 4:KC // 2],
                                                op=mybir.AluOpType.max)
                        s3 = pool.tile([P, KC // 8], BF16, tag="s3")
                        nc.vector.tensor_add(out=s3[:], in0=s2[:, 0:KC // 8],
                                             in1=s2[:, KC // 8:KC // 4])
                        m3 = pool.tile([P, KC // 8], BF16, tag="m3")
                        nc.vector.tensor_tensor(out=m3[:], in0=m2[:, 0:KC // 8],
                                                in1=m2[:, KC // 8:KC // 4],
                                                op=mybir.AluOpType.max)
                    pm = pool.tile([P, C], F32, tag="pm")
                    nc.vector.tensor_add(out=pm[:], in0=s3[:, 0:C],
                                         in1=s3[:, C:2 * C])
                    px = pool.tile([P, C], F32, tag="px")
                    nc.vector.tensor_tensor(out=px[:], in0=m3[:, 0:C],
                                            in1=m3[:, C:2 * C],
                                            op=mybir.AluOpType.max)
                    # transpose both pools to [C, P]
                    ps_m = ppool.tile([C, P], F32, tag="tp1")
                    nc.tensor.transpose(out=ps_m[:], in_=pm[:], identity=idt[:])
                    ps_x = ppool.tile([C, P], F32, tag="tp2")
                    nc.tensor.transpose(out=ps_x[:], in_=px[:], identity=idt[:])
                    poolT = pool.tile([C, 2 * P], F32, tag="poolT")
                    # mean = sum/16 folded into the copy
                    nc.scalar.activation(out=poolT[:, 0:P], in_=ps_m[:],
                                         func=mybir.ActivationFunctionType.Copy,
                                         scale=1.0 / K)
                    nc.scalar.activation(out=poolT[:, P:2 * P], in_=ps_x[:],
                                         func=mybir.ActivationFunctionType.Copy)
                    ps1 = ppool2.tile([HID, 2 * P], F32, tag="mm1")
                    nc.tensor.matmul(out=ps1[:], lhsT=w1s[:], rhs=poolT[:],
                                     start=True, stop=True)
                    h = pool.tile([HID, 2 * P], F32, tag="h")
                    nc.scalar.activation(out=h[:], in_=ps1[:],
                                         func=mybir.ActivationFunctionType.Relu,
                                         bias=b1s[:])
                    # accumulate mean- and max-path MLP outputs directly in PSUM
                    ps2 = ppool2.tile([C, P], F32, tag="mm2")
                    nc.tensor.matmul(out=ps2[:], lhsT=w2s[:], rhs=h[:, 0:P],
                                     start=True, stop=False)
                    nc.tensor.matmul(out=ps2[:], lhsT=w2s[:], rhs=h[:, P:2 * P],
                                     start=False, stop=True)
                    gT = pool.tile([C, P], F32, tag="gT")
                    nc.scalar.activation(out=gT[:], in_=ps2[:],
                                         func=mybir.ActivationFunctionType.Sigmoid,
                                         bias=b2s[:])
                    psg = ppool.tile([P, C], F32, tag="tpg")
                    nc.tensor.transpose(out=psg[:], in_=gT[:],
                                        identity=idt[0:C, 0:C])
                    gate = pool.tile([P, C], F32, tag="gate")
                    nc.scalar.activation(out=gate[:], in_=psg[:],
                                         func=mybir.ActivationFunctionType.Copy)
                    xt = pool.tile([P, C], F32, tag="xt")
                    nc.sync.dma_start(out=xt[:], in_=xo[r0:r0 + P, :])
                    ot = pool.tile([P, C], F32, tag="ot")
                    s0 = pool.tile([P, 1], F32, tag="s0")
                    nc.vector.tensor_mul(out=ot[:], in0=xt[:], in1=gate[:])
                    nc.vector.tensor_reduce(out=s0[:], in_=ot[:],
                                            axis=mybir.AxisListType.X,
                                            op=mybir.AluOpType.add)
                    nc.sync.dma_start(out=outse[r0:r0 + P, :], in_=ot[:])
                    smt = pool.tile([P, 2], BF16, tag="smt")
                    nc.scalar.activation(out=smt[:, 0:1], in_=s0[:],
                                         func=mybir.ActivationFunctionType.Copy,
                                         scale=1.0 / C)
                    nc.vector.tensor_reduce(out=smt[:, 1:2], in_=ot[:],
                                            axis=mybir.AxisListType.X,
                                            op=mybir.AluOpType.max)
                    nc.sync.dma_start(out=sm[r0:r0 + P, :], in_=smt[:])
    nc.compile()
    return nc


def build_l2(repeat=1):
    nc = _nc()
    smf = nc.dram_tensor("smf", [SMF_ROWS, 2], BF16, kind="ExternalInput")
    ji2 = nc.dram_tensor("ji2", [SHP, K], I32, kind="ExternalInput")
    z = nc.dram_tensor("z", [SHP, 2], F32, kind="ExternalOutput")
    with TileContext(nc) as tc:
        with tc.tile_pool(name="sbuf", bufs=4) as pool, \
             tc.tile_pool(name="ipool", bufs=4) as ipool:
            rep_ctx = ExitStack()
            if repeat > 1:
                rep_ctx.enter_context(tc.For_i(0, repeat, 1))
            with rep_ctx:
                for t in range(NT):
                    r0 = t * P
                    it = ipool.tile([P, K], I32, tag="it")
                    nc.sync.dma_start(out=it[:], in_=ji2[r0:r0 + P, :])
                    sg = pool.tile([P, K * 2], BF16, tag="sg")
                    for j in range(K):
                        nc.gpsimd.indirect_dma_start(
                            out=sg[:, j * 2:(j + 1) * 2], out_offset=None,
                            in_=smf[:],
                            in_offset=bass.IndirectOffsetOnAxis(
                                ap=it[:, j:j + 1], axis=0),
                        )
                    sv = sg[:].rearrange("p (j c) -> p c j", j=K)
                    rs = pool.tile([P, 2], F32, tag="rs")
                    nc.vector.tensor_reduce(out=rs[:], in_=sv,
                                            axis=mybir.AxisListType.X,
                                            op=mybir.AluOpType.add)
                    rm = pool.tile([P, 2], F32, tag="rm")
                    nc.vector.tensor_reduce(out=rm[:], in_=sv,
                                            axis=mybir.AxisListType.X,
                                            op=mybir.AluOpType.max)
                    zt = pool.tile([P, 2], F32, tag="zt")
                    nc.scalar.activation(out=zt[:, 0:1], in_=rs[:, 0:1],
                                         func=mybir.ActivationFunctionType.Copy,
                                         scale=1.0 / K)
                    nc.vector.tensor_copy(out=zt[:, 1:2], in_=rm[:, 1:2])
                    nc.sync.dma_start(out=z[r0:r0 + P, :], in_=zt[:])
    nc.compile()
    return nc


def build_l3(repeat=1, emax=EMAX):
    nc = _nc()
    zf = nc.dram_tensor("zf", [ZF_ROWS, 2], F32, kind="ExternalInput")
    cwt = nc.dram_tensor("cwt", [32, 2], F32, kind="ExternalInput")
    ei2 = nc.dram_tensor("ei2", [SHP, 2 * emax], I32, kind="ExternalInput")
    oi = nc.dram_tensor("oi", [SHP, C], F32, kind="ExternalInput")
    out = nc.dram_tensor("out", [SHP, C], F32, kind="ExternalOutput")

    with TileContext(nc) as tc:
        with tc.tile_pool(name="sbuf", bufs=4) as pool, \
             tc.tile_pool(name="ipool", bufs=4) as ipool:
            rep_ctx = ExitStack()
            if repeat > 1:
                rep_ctx.enter_context(tc.For_i(0, repeat, 1))
            with rep_ctx:
                for t in range(NT):
                    r0 = t * P
                    et = ipool.tile([P, 2 * emax], I32, tag="et")
                    nc.sync.dma_start(out=et[:], in_=ei2[r0:r0 + P, :])
                    zn = pool.tile([P, emax * 2], F32, tag="zn")
                    wn = pool.tile([P, emax * 2], F32, tag="wn")
                    for e in range(emax):
                        nc.gpsimd.indirect_dma_start(
                            out=zn[:, e * 2:(e + 1) * 2], out_offset=None,
                            in_=zf[:],
                            in_offset=bass.IndirectOffsetOnAxis(
                                ap=et[:, e:e + 1], axis=0),
                        )
                        nc.gpsimd.indirect_dma_start(
                            out=wn[:, e * 2:(e + 1) * 2], out_offset=None,
                            in_=cwt[:],
                            in_offset=bass.IndirectOffsetOnAxis(
                                ap=et[:, emax + e:emax + e + 1], axis=0),
                        )
                    pr = pool.tile([P, emax * 2], F32, tag="pr")
                    cf = pool.tile([P, 1], F32, tag="cf")
                    nc.vector.tensor_mul(out=pr[:], in0=zn[:], in1=wn[:])
                    nc.vector.tensor_reduce(out=cf[:], in_=pr[:],
                                            axis=mybir.AxisListType.X,
                                            op=mybir.AluOpType.add)
                    sg = pool.tile([P, 1], F32, tag="sig")
                    nc.scalar.activation(out=sg[:], in_=cf[:],
                                         func=mybir.ActivationFunctionType.Sigmoid)
                    ot = pool.tile([P, C], F32, tag="ot")
                    nc.sync.dma_start(out=ot[:], in_=oi[r0:r0 + P, :])
                    ft = pool.tile([P, C], F32, tag="ft")
                    nc.vector.tensor_mul(out=ft[:], in0=ot[:],
                                         in1=sg[:].to_broadcast([P, C]))
                    nc.sync.dma_start(out=out[r0:r0 + P, :], in_=ft[:])
    nc.compile()
    return nc


def build_l3_csr(repeat=1, m3=3):
    """Sparse-conv via CSR-packed slot gathers + indicator matmul.

    Per 128-point tile: m3 indirect calls each gather 128 z-pairs (one per
    partition); per-slot weight pairs are host-uploaded; a 0/1 indicator
    matmul on PE segment-sums the weighted slot dots into per-point convf.
    """
    nc = _nc()
    zf = nc.dram_tensor("zf", [ZF_ROWS, 2], F32, kind="ExternalInput")
    ei = nc.dram_tensor("ei", [SHP, m3], I32, kind="ExternalInput")
    wv = nc.dram_tensor("wv", [SHP, 2 * m3], F32, kind="ExternalInput")
    ind = nc.dram_tensor("ind", [SHP, m3 * P], BF16, kind="ExternalInput")
    oi = nc.dram_tensor("oi", [SHP, C], F32, kind="ExternalInput")
    out = nc.dram_tensor("out", [SHP, C], F32, kind="ExternalOutput")

    with TileContext(nc) as tc:
        with tc.tile_pool(name="sbuf", bufs=4) as pool, \
             tc.tile_pool(name="ipool", bufs=4) as ipool, \
             tc.tile_pool(name="psum", bufs=2, space="PSUM") as ppool:
            rep_ctx = ExitStack()
            if repeat > 1:
                rep_ctx.enter_context(tc.For_i(0, repeat, 1))
            with rep_ctx:
                for t in range(NT):
                    r0 = t * P
                    et = ipool.tile([P, m3], I32, tag="et")
                    nc.sync.dma_start(out=et[:], in_=ei[r0:r0 + P, :])
                    wt = pool.tile([P, 2 * m3], F32, tag="wt")
                    nc.sync.dma_start(out=wt[:], in_=wv[r0:r0 + P, :])
                    it = pool.tile([P, m3 * P], BF16, tag="ind")
                    nc.sync.dma_start(out=it[:], in_=ind[r0:r0 + P, :])
                    zn = pool.tile([P, 2 * m3], F32, tag="zn")
                    for k in range(m3):
                        nc.gpsimd.indirect_dma_start(
                            out=zn[:, 2 * k:2 * k + 2], out_offset=None,
                            in_=zf[:],
                            in_offset=bass.IndirectOffsetOnAxis(
                                ap=et[:, k:k + 1], axis=0),
                        )
                    pr = pool.tile([P, 2 * m3], F32, tag="pr")
                    nc.vector.tensor_mul(out=pr[:], in0=zn[:], in1=wt[:])
                    with nc.allow_low_precision("slot dot in bf16 for matmul"):
                        u = pool.tile([P, m3], BF16, tag="u")
                        nc.vector.tensor_reduce(
                            out=u[:], in_=pr[:].rearrange("p (k c) -> p k c",
                                                          k=m3),
                            axis=mybir.AxisListType.X, op=mybir.AluOpType.add)
                    psc = ppool.tile([P, 1], F32, tag="cf")
                    for k in range(m3):
                        nc.tensor.matmul(out=psc[:],
                                         lhsT=it[:, k * P:(k + 1) * P],
                                         rhs=u[:, k:k + 1],
                                         start=(k == 0), stop=(k == m3 - 1))
                    sg = pool.tile([P, 1], F32, tag="sig")
                    nc.scalar.activation(out=sg[:], in_=psc[:],
                                         func=mybir.ActivationFunctionType.Sigmoid)
                    ot = pool.tile([P, C], F32, tag="ot")
                    nc.sync.dma_start(out=ot[:], in_=oi[r0:r0 + P, :])
                    ft = pool.tile([P, C], F32, tag="ft")
                    nc.vector.tensor_mul(out=ft[:], in0=ot[:],
                                         in1=sg[:].to_broadcast([P, C]))
                    nc.sync.dma_start(out=out[r0:r0 + P, :], in_=ft[:])
    nc.compile()
    return nc


def csr_prep(conv_idx_shard, conv_w, m3=None):
    """Pack valid (point, tap) slots of one shard into m3 128-slot calls per
    128-point tile.  Returns (ei [SHP, m3], wv [SHP, 2*m3],
    ind [NT*m3*128, 128] bf16, m3)."""
    import ml_dtypes
    qmap = lambda n: (n // SH) * SHP + (n % SH)          # noqa: E731
    valid = conv_idx_shard >= 0                          # [SH(P?), 27]
    npts = conv_idx_shard.shape[0]
    cnt_t = np.array([valid[t * P:(t + 1) * P].sum() for t in range(NT)])
    if m3 is None:
        m3 = max(int(np.ceil(cnt_t.max() / P)), 1)
    ei = np.full((SHP, m3), SH, np.int64)   # default: zero pad row of shard 0
    wv = np.zeros((SHP, 2 * m3), np.float32)
    ind = np.zeros((SHP, m3 * P), np.float32)
    w2 = conv_w.reshape(27, 2)
    for t in range(NT):
        pv, lv = np.nonzero(valid[t * P:(t + 1) * P])    # slot lists (sorted by p)
        nb = conv_idx_shard[t * P + pv, lv]
        s = np.arange(len(pv))
        part, col = s % P, s // P                        # slot -> (partition, call)
        ei[t * P + part, col] = qmap(nb.astype(np.int64))
        wv[t * P + part, 2 * col] = w2[lv, 0]
        wv[t * P + part, 2 * col + 1] = w2[lv, 1]
        ind[t * P + part, col * P + pv] = 1.0
    return (ei.astype(np.int32), wv,
            ind.astype(ml_dtypes.bfloat16), m3)


def _pad_rows(a, rows):
    out = np.zeros((rows,) + a.shape[1:], a.dtype)
    out[:a.shape[0]] = a
    return out


def kernel(x_F, W1, b1, W2, b2, conv_w, idx, conv_idx):
    x_F = np.ascontiguousarray(np.asarray(x_F, dtype=np.float32))
    W1 = np.asarray(W1, dtype=np.float32)
    b1 = np.asarray(b1, dtype=np.float32)
    W2 = np.asarray(W2, dtype=np.float32)
    b2 = np.asarray(b2, dtype=np.float32)
    conv_w = np.asarray(conv_w, dtype=np.float32)
    idx = np.asarray(idx).astype(np.int32)
    conv_idx = np.asarray(conv_idx).astype(np.int32)

    import ml_dtypes
    xf_b = x_F.astype(ml_dtypes.bfloat16)
    cores = list(range(NCORES))
    # host-side index prep (pure index transforms)
    qmap = lambda n: (n // SH) * SHP + (n % SH)          # noqa: E731
    idx_q = qmap(idx.astype(np.int64)).astype(np.int32)  # shard-concat ids
    valid = conv_idx >= 0
    ci_q = np.where(valid, qmap(np.clip(conv_idx, 0, None).astype(np.int64)),
                    0).astype(np.int32)
    lidx = np.broadcast_to(np.arange(27, dtype=np.int32), conv_idx.shape)
    lrow = np.where(valid, lidx, 27).astype(np.int32)    # 27 -> zero weight row
    # compact valid entries to the first emax slots per row
    cnt = valid.sum(1)
    emax = max(int(cnt.max()), 1)
    order = np.argsort(~valid, axis=1, kind="stable")    # valid first
    eiz = np.take_along_axis(ci_q, order, axis=1)[:, :emax]
    eil = np.take_along_axis(lrow, order, axis=1)[:, :emax]
    ei2 = np.ascontiguousarray(np.concatenate([eiz, eil], axis=1))

    # ---- L1
    nc1 = build_l1()
    in1 = []
    for c in cores:
        sl = slice(c * SH, (c + 1) * SH)
        in1.append({
            "xf": xf_b,
            "xo": _pad_rows(x_F[sl], SHP),
            "ji": _pad_rows(idx[sl], SHP),
            "w1": W1,
            "b1": b1.reshape(HID, 1),
            "w2": W2,
            "b2x2": (2.0 * b2).reshape(C, 1),
        })
    r1 = _run(nc1, in1, "L1")
    outse = np.concatenate([r1.results[c]["outse"] for c in cores], 0)
    smf = np.concatenate([r1.results[c]["sm"] for c in cores], 0)

    # ---- L2
    nc2 = build_l2()
    in2 = []
    for c in cores:
        sl = slice(c * SH, (c + 1) * SH)
        in2.append({"smf": smf, "ji2": _pad_rows(idx_q[sl], SHP)})
    r2 = _run(nc2, in2, "L2")
    zf = np.zeros((ZF_ROWS, 2), np.float32)
    for c in cores:
        zc = r2.results[c]["z"]
        zf[c * SHP: c * SHP + SH] = zc[:SH]   # zero the pad rows

    # ---- L3 (CSR-packed slots + indicator matmul)
    preps = [csr_prep(conv_idx[c * SH:(c + 1) * SH], conv_w) for c in cores]
    m3 = max(p[3] for p in preps)
    if any(p[3] != m3 for p in preps):
        preps = [csr_prep(conv_idx[c * SH:(c + 1) * SH], conv_w, m3)
                 for c in cores]
    nc3 = build_l3_csr(m3=m3)
    in3 = []
    for c in cores:
        ei3, wv3, ind3, _ = preps[c]
        in3.append({
            "zf": zf,
            "ei": ei3,
            "wv": wv3,
            "ind": ind3,
            "oi": outse[c * SHP:(c + 1) * SHP],
        })
    r3 = _run(nc3, in3, "L3")
    out = np.concatenate([r3.results[c]["out"][:SH] for c in cores], 0)
    return out
